# revision 6
# baseline (speedup 1.0000x reference)
"""Ragged masked-softmax attention-energy kernel for 8 Trainium2 NeuronCores.

Reference computation (B2=512, L=1024, E=512):
    energy = questions @ W.T + b              [B2, L, E]
    scores = energy @ weight_vec              [B2, L]
    scores[l >= len] = -inf
    out = softmax(scores, axis=1)

Algebraic facts that make this a pure HBM-streaming problem:
  * (q @ W.T + b) @ wv == q @ (W.T @ wv) + (b . wv); softmax is shift
    invariant so the (b . wv) scalar cancels. Only v = W.T @ wv (a [E]
    vector, computed on device) ever multiplies the big tensor.
  * tokens at positions >= len contribute exactly 0 to the output, so
    only the first `len` tokens of each row need to be loaded at all.

The big tensor is packed in bf16 (tolerance is 2e-2; bf16 costs ~0.1%
of a prob) and TRANSPOSED on the host into [e, tok] column tiles of
EXACT width (no padding to 128): each column holds w <= 128 tokens of
one row, stored as 4 e-chunks of [128, w]. The per-token dot products
run on the TensorEngine: per column, four accumulating matmuls with the
q e-chunk as the (FWL-eligible bf16) stationary operand and the
matching vT chunk as a 1-column moving operand. Scores land directly as
[tok, col] fp32 PSUM blocks; the tail per 128-column block is mask add,
PE transpose, ACT exp with free-axis accumulate (per-col sums), and 0/1
segment matmuls that map columns to rows entirely as data, keeping the
SPMD program identical on all 8 cores. Column slot widths are the
elementwise max over cores of each core's width-sorted column list, so
the program is uniform while wasting <1% over the exact token count.

Host side is pure data layout (transpose/cast/scatter, no math): rows
are LPT-packed across cores by exact token count; host scatters the
packed [col, 128] probabilities back into the [B2, L] output.
"""

import os
import sys

import numpy as np
import ml_dtypes

if "/opt/trn_rl_repo" not in sys.path:
    sys.path.insert(0, "/opt/trn_rl_repo")

E = 512
P = 128
CH = E // P  # e-chunks per tile
WMAX = 8192  # free-dim elements per DMA group (= 2 MiB bf16 per group)
NCORES = 8
NEG = -1.0e30
BF16 = ml_dtypes.bfloat16

_NC_CACHE = {}
LAST_RESULT = None


def _schedule(lens, n_cores):
    """Assign rows to cores (LPT by exact token count, <=128 rows/core)."""
    order = sorted(range(len(lens)), key=lambda r: -int(lens[r]))
    loads = [0] * n_cores
    rows_of = [[] for _ in range(n_cores)]
    for r in order:
        cands = [c for c in range(n_cores) if len(rows_of[c]) < P]
        c = min(cands, key=lambda i: (loads[i], len(rows_of[i])))
        rows_of[c].append(r)
        loads[c] += int(lens[r])
    return rows_of


def _layout(lens, n_cores):
    """Uniform SPMD layout: per-core width-sorted columns, slot widths =
    elementwise max over cores, greedy grouping into <=WMAX DMA groups.

    Returns (slots, Wg, NB, cols_of): slots[i] = (g, off, w); Wg[g] =
    used free width of group g; cols_of[c][i] = (row, tile, ntok) or
    None for this core's dummy slots.
    """
    rows_of = _schedule(lens, n_cores)
    percore = []
    for c in range(n_cores):
        cols = []
        for r in rows_of[c]:
            L = int(lens[r])
            for t in range((L + P - 1) // P):
                cols.append((min(P, L - t * P), r, t))
        cols.sort(key=lambda x: (-x[0], x[1], x[2]))
        percore.append(cols)
    ncol = max(len(c) for c in percore)
    wslot = []
    for i in range(ncol):
        w = max((c[i][0] if i < len(c) else 0) for c in percore)
        wslot.append(min(P, ((w + 1) // 2) * 2))
    slots = []
    Wg = []
    cur = 0
    for i, w in enumerate(wslot):
        need = CH * w
        if cur + need > WMAX:
            Wg.append(cur)
            cur = 0
        slots.append((len(Wg), cur, w))
        cur += need
    Wg.append(cur)
    NB = -(-ncol // P)
    cols_of = [[(c[i][1], c[i][2], c[i][0]) if i < len(c) else None
                for i in range(ncol)] for c in percore]
    return tuple(slots), tuple(Wg), NB, cols_of


def _pack(questions, lens, n_cores):
    B2, L, E_ = questions.shape
    assert E_ == E
    slots, Wg, NB, cols_of = _layout(lens, n_cores)
    ncol = len(slots)
    COLS = NB * P
    SUMW = sum(Wg)
    off_g = np.concatenate([[0], np.cumsum(Wg)]).astype(np.int64)
    in_maps = []
    for c in range(n_cores):
        qp = np.zeros((P, SUMW), BF16)
        msk = np.full((P, COLS), NEG, np.float32)
        seg = np.zeros((P, COLS), np.float32)
        segT = np.zeros((P, COLS), np.float32)
        local = {}
        for s in range(ncol):
            if cols_of[c][s] is None:
                continue
            r, t, ntok = cols_of[c][s]
            g, off, w = slots[s]
            base = off_g[g] + off
            qt = questions[r, t * P:t * P + ntok, :]  # [ntok, E] fp32
            for j in range(CH):  # [e_in_chunk, tok] per chunk at stride w
                qp[:, base + j * w:base + j * w + ntok] = qt[:, j * P:(j + 1) * P].T
            msk[:ntok, s] = 0.0
            li = local.setdefault(r, len(local))
            b_, m = divmod(s, P)
            seg[m, b_ * P + li] = 1.0
            segT[li, b_ * P + m] = 1.0
        in_maps.append({"qp": qp, "msk": msk, "seg": seg, "segT": segT})
    return in_maps, cols_of, (slots, Wg, NB)


def _build_nc(layout, reps=1):
    from concourse import bacc, bass, tile

    mybir = bass.mybir
    dt = mybir.dt.float32
    dtb = mybir.dt.bfloat16
    Alu = mybir.AluOpType
    ActF = mybir.ActivationFunctionType
    slots, Wg, NB = layout
    ncol = len(slots)
    COLS = NB * P
    SUMW = sum(Wg)
    off_g = [0]
    for w in Wg:
        off_g.append(off_g[-1] + w)

    nc = bacc.Bacc("TRN2", target_bir_lowering=False, debug=False,
                   num_devices=NCORES)
    qp = nc.declare_dram_parameter("qp", [P, SUMW], dtb, isOutput=False)
    msk = nc.declare_dram_parameter("msk", [P, COLS], dt, isOutput=False)
    seg = nc.declare_dram_parameter("seg", [P, COLS], dt, isOutput=False)
    segT = nc.declare_dram_parameter("segT", [P, COLS], dt, isOutput=False)
    iden = nc.declare_dram_parameter("iden", [P, P], dt, isOutput=False)
    wm = nc.declare_dram_parameter("wm", [E, E], dt, isOutput=False)
    wv = nc.declare_dram_parameter("wv", [4, P], dt, isOutput=False)
    # shape varies with reps so the jax persistent compile cache cannot
    # alias NEFFs of different-reps builds (the BIR is not in the HLO key)
    nc.declare_dram_parameter("stamp", [1, reps], dt, isOutput=False)
    probs = nc.declare_dram_parameter("probs", [COLS, P], dt, isOutput=True)

    with tile.TileContext(nc) as tc:
        with (
            tc.tile_pool(name="const", bufs=1) as const,
            tc.tile_pool(name="qpool", bufs=5) as qpool,
            tc.tile_pool(name="spool", bufs=2) as spool,
            tc.tile_pool(name="scratch", bufs=2) as scratch,
            tc.tile_pool(name="ppool", bufs=1) as ppool,
            tc.tile_pool(name="psum", bufs=1, space=bass.MemorySpace.PSUM) as psp,
            tc.tile_pool(name="psum2", bufs=2, space=bass.MemorySpace.PSUM) as psp2,
            tc.tile_pool(name="psc", bufs=2, space=bass.MemorySpace.PSUM) as psc,
        ):
            iden_sb = const.tile([P, P], dt, tag="iden")
            nc.sync.dma_start(iden_sb[:], iden[:])
            msk_sb = const.tile([P, COLS], dt, tag="msk")
            nc.sync.dma_start(msk_sb[:], msk[:])
            seg_sb = const.tile([P, COLS], dt, tag="seg")
            nc.sync.dma_start(seg_sb[:], seg[:])
            segT_sb = const.tile([P, COLS], dt, tag="segT")
            nc.sync.dma_start(segT_sb[:], segT[:])
            w_sb = const.tile([P, 4 * E], dt, tag="wmat")
            for j in range(4):
                nc.sync.dma_start(w_sb[:, j * E:(j + 1) * E],
                                  wm[j * P:(j + 1) * P, :])
            wv4 = const.tile([4, P], dt, tag="wv4")
            nc.sync.dma_start(wv4[:], wv[:])

            # wvT[f_in_chunk, fchunk] = wv[f] on 128 partitions
            wvT_ps = psp.tile([P, 4], dt, tag="setup")
            nc.tensor.transpose(wvT_ps[:], wv4[:], iden_sb[0:4, 0:4])
            wvT_sb = const.tile([P, 4], dt, tag="wvT")
            nc.scalar.copy(wvT_sb[:], wvT_ps[:])
            # vT[e_in_chunk, echunk] = (W.T @ wv)[e], via 4x4 accumulating
            # matvecs: out[m, c] = sum_f W[f, c*128+m] * wv[f]
            vT_ps = psp.tile([P, CH], dt, tag="setup")
            with tc.tile_critical():
                for cch in range(CH):
                    for j in range(4):
                        nc.tensor.matmul(
                            vT_ps[:, cch:cch + 1],
                            w_sb[:, j * E + cch * P:j * E + (cch + 1) * P],
                            wvT_sb[:, j:j + 1],
                            start=(j == 0), stop=(j == 3))
            vT_bf = const.tile([P, CH], dtb, tag="vTbf")
            nc.vector.tensor_copy(vT_bf[:], vT_ps[:])

            def one_pass():
                rowsum_parts = const.tile([P, NB], dt, tag="rsparts")
                pr_tiles = []
                live = {}
                for b in range(NB):
                    sc_ps = psc.tile([P, P], dt, tag="scps")
                    # dummy cols / tokens past a col's width keep NEG; a
                    # col's matmul group overwrites only rows [0, w)
                    nc.vector.memset(sc_ps[:], NEG)
                    for s in range(b * P, min((b + 1) * P, ncol)):
                        g, off, w = slots[s]
                        cl = s - b * P
                        if g not in live:
                            qt = qpool.tile([P, WMAX], dtb, tag="q")
                            nc.sync.dma_start(qt[:, :Wg[g]],
                                              qp[:, off_g[g]:off_g[g + 1]])
                            live[g] = qt
                        qt = live[g]
                        for j in range(CH):
                            nc.tensor.matmul(
                                sc_ps[0:w, cl:cl + 1],
                                qt[:, off + j * w:off + (j + 1) * w],
                                vT_bf[:, j:j + 1],
                                start=(j == 0), stop=(j == CH - 1))
                    # apply length/padding mask
                    sc2_b = spool.tile([P, P], dt, tag="scores2")
                    nc.vector.tensor_tensor(
                        out=sc2_b[:], in0=sc_ps[:],
                        in1=msk_sb[:, b * P:(b + 1) * P], op=Alu.add)
                    # block tail: transpose -> exp(+sum) -> segment row-sums
                    # (walrus: transpose matmul output must start at PSUM
                    # partition 0, so each 64-col chunk gets its own tile)
                    pr_b = ppool.tile([P, P], dt, tag=f"pr{b}")
                    se_b = ppool.tile([P, 1], dt, tag=f"se{b}")
                    for i in range(2):
                        pt_ps = psp2.tile([64, P], dt, tag="tps")
                        nc.tensor.transpose(pt_ps[:],
                                            sc2_b[:, 64 * i:64 * (i + 1)],
                                            iden_sb[:])
                        nc.scalar.activation(
                            pr_b[64 * i:64 * (i + 1), :],
                            pt_ps[:],
                            ActF.Exp,
                            accum_out=se_b[64 * i:64 * (i + 1), 0:1])
                    rs_ps = psp2.tile([P, 1], dt, tag="v1")
                    nc.tensor.matmul(rs_ps[:], seg_sb[:, b * P:(b + 1) * P],
                                     se_b[:], start=True, stop=True)
                    nc.scalar.copy(rowsum_parts[:, b:b + 1], rs_ps[:])
                    pr_tiles.append(pr_b)

                rowsum = const.tile([P, 1], dt, tag="rowsum")
                nc.vector.reduce_sum(rowsum[:], rowsum_parts[:],
                                     axis=mybir.AxisListType.X)
                rowsum_eps = const.tile([P, 1], dt, tag="rowsum_eps")
                # keep unused-row reciprocals finite so 0-weight matmul
                # terms stay 0 instead of 0*inf
                nc.vector.tensor_scalar_add(rowsum_eps[:], rowsum[:], 1e-30)
                recip_sb = const.tile([P, 1], dt, tag="recip")
                nc.vector.reciprocal(recip_sb[:], rowsum_eps[:])

                for b in range(NB):
                    ex_ps = psp2.tile([P, 1], dt, tag="v1")
                    nc.tensor.matmul(ex_ps[:], segT_sb[:, b * P:(b + 1) * P],
                                     recip_sb[:], start=True, stop=True)
                    ex_sb = scratch.tile([P, 1], dt, tag="exs")
                    nc.scalar.copy(ex_sb[:], ex_ps[:])
                    fin = scratch.tile([P, P], dt, tag="fin")
                    nc.vector.tensor_scalar_mul(fin[:], pr_tiles[b][:],
                                                ex_sb[:])
                    nc.sync.dma_start(probs[b * P:(b + 1) * P, :], fin[:])

            for _rep in range(reps):
                one_pass()

    nc.compile()
    return nc


def kernel(**inputs):
    global LAST_RESULT
    from concourse.bass_utils import run_bass_kernel_spmd

    questions = np.ascontiguousarray(np.asarray(inputs["questions"], np.float32))
    lens = np.asarray(inputs["questions_lens"], np.int32)
    W = np.ascontiguousarray(np.asarray(inputs["W"], np.float32))
    wv = np.ascontiguousarray(np.asarray(inputs["weight_vec"], np.float32))
    B2, L, E_ = questions.shape

    in_maps, cols_of, layout = _pack(questions, lens, NCORES)
    iden = np.eye(P, dtype=np.float32)
    wvr = np.ascontiguousarray(wv.reshape(4, P))
    for m in in_maps:
        m["iden"] = iden
        m["wm"] = W
        m["wv"] = wvr
        m["stamp"] = np.zeros((1, 1), np.float32)

    if layout not in _NC_CACHE:
        _NC_CACHE[layout] = _build_nc(layout)
    nc = _NC_CACHE[layout]

    res = run_bass_kernel_spmd(nc, in_maps, list(range(NCORES)))
    LAST_RESULT = res

    out = np.zeros((B2, L), np.float32)
    for c in range(NCORES):
        pr = res.results[c]["probs"]
        for s, col in enumerate(cols_of[c]):
            if col is None:
                continue
            r, t, ntok = col
            out[r, t * P:t * P + ntok] = pr[s, :ntok]
    return out


# revision 9
# speedup vs baseline: 1.2998x; 1.2998x over previous
"""Ragged masked-softmax attention-energy kernel for 8 Trainium2 NeuronCores.

Reference computation (B2=512, L=1024, E=512):
    energy = questions @ W.T + b              [B2, L, E]
    scores = energy @ weight_vec              [B2, L]
    scores[l >= len] = -inf
    out = softmax(scores, axis=1)

Algebraic facts that make this a pure HBM-streaming problem:
  * (q @ W.T + b) @ wv == q @ (W.T @ wv) + (b . wv); softmax is shift
    invariant so the (b . wv) scalar cancels. Only v = W.T @ wv (a [E]
    vector, computed on device) ever multiplies the big tensor.
  * tokens at positions >= len contribute exactly 0 to the output, so
    only the first `len` tokens of each row need to be loaded at all.

The big tensor is packed in bf16 (tolerance is 2e-2; bf16 costs ~0.1%
of a prob) and TRANSPOSED on the host into [e, tok] column tiles of
EXACT width (no padding to 128): each column holds w <= 128 tokens of
one row, stored as 4 e-chunks of [128, w]. The per-token dot products
run on the TensorEngine: per column, four accumulating matmuls with the
q e-chunk as the (FWL-eligible bf16) stationary operand and the
matching vT chunk as a 1-column moving operand. Scores land directly as
[tok, col] fp32 PSUM blocks; the tail per 128-column block is mask add,
PE transpose, ACT exp with free-axis accumulate (per-col sums), and 0/1
segment matmuls that map columns to rows entirely as data, keeping the
SPMD program identical on all 8 cores. Column slot widths are the
elementwise max over cores of each core's width-sorted column list, so
the program is uniform while wasting <1% over the exact token count.

Host side is pure data layout (transpose/cast/scatter, no math): rows
are LPT-packed across cores by exact token count; host scatters the
packed [col, 128] probabilities back into the [B2, L] output.
"""

import os
import sys

import numpy as np
import ml_dtypes

if "/opt/trn_rl_repo" not in sys.path:
    sys.path.insert(0, "/opt/trn_rl_repo")

E = 512
P = 128
CH = E // P  # e-chunks per tile
WMAX = 8192  # free-dim elements per DMA group (= 2 MiB bf16 per group)
NCORES = 8
NEG = -1.0e30
BF16 = ml_dtypes.bfloat16

_NC_CACHE = {}
LAST_RESULT = None


def _schedule(lens, n_cores):
    """Assign rows to cores (LPT by exact token count, <=128 rows/core)."""
    order = sorted(range(len(lens)), key=lambda r: -int(lens[r]))
    loads = [0] * n_cores
    rows_of = [[] for _ in range(n_cores)]
    for r in order:
        cands = [c for c in range(n_cores) if len(rows_of[c]) < P]
        c = min(cands, key=lambda i: (loads[i], len(rows_of[i])))
        rows_of[c].append(r)
        loads[c] += int(lens[r])
    return rows_of


def _layout(lens, n_cores):
    """Uniform SPMD layout: per-core width-sorted columns, slot widths =
    elementwise max over cores, greedy grouping into <=WMAX DMA groups.

    Returns (slots, Wg, NB, cols_of): slots[i] = (g, off, w); Wg[g] =
    used free width of group g; cols_of[c][i] = (row, tile, ntok) or
    None for this core's dummy slots.
    """
    rows_of = _schedule(lens, n_cores)
    percore = []
    for c in range(n_cores):
        cols = []
        for r in rows_of[c]:
            L = int(lens[r])
            for t in range((L + P - 1) // P):
                cols.append((min(P, L - t * P), r, t))
        cols.sort(key=lambda x: (-x[0], x[1], x[2]))
        percore.append(cols)
    ncol = max(len(c) for c in percore)
    wslot = []
    for i in range(ncol):
        w = max((c[i][0] if i < len(c) else 0) for c in percore)
        wslot.append(min(P, ((w + 1) // 2) * 2))
    slots = []
    Wg = []
    cur = 0
    for i, w in enumerate(wslot):
        need = CH * w
        if cur + need > WMAX:
            Wg.append(cur)
            cur = 0
        slots.append((len(Wg), cur, w))
        cur += need
    Wg.append(cur)
    NB = -(-ncol // P)
    cols_of = [[(c[i][1], c[i][2], c[i][0]) if i < len(c) else None
                for i in range(ncol)] for c in percore]
    return tuple(slots), tuple(Wg), NB, cols_of


def _pack(questions, lens, n_cores):
    B2, L, E_ = questions.shape
    assert E_ == E
    slots, Wg, NB, cols_of = _layout(lens, n_cores)
    ncol = len(slots)
    COLS = NB * P
    SUMW = sum(Wg)
    off_g = np.concatenate([[0], np.cumsum(Wg)]).astype(np.int64)
    in_maps = []
    for c in range(n_cores):
        qp = np.zeros((P, SUMW), BF16)
        msk = np.full((P, COLS), NEG, np.float32)
        seg = np.zeros((P, COLS), np.float32)
        segT = np.zeros((P, COLS), np.float32)
        local = {}
        for s in range(ncol):
            if cols_of[c][s] is None:
                continue
            r, t, ntok = cols_of[c][s]
            g, off, w = slots[s]
            base = off_g[g] + off
            qt = questions[r, t * P:t * P + ntok, :]  # [ntok, E] fp32
            for j in range(CH):  # [e_in_chunk, tok] per chunk at stride w
                qp[:, base + j * w:base + j * w + ntok] = qt[:, j * P:(j + 1) * P].T
            msk[:ntok, s] = 0.0
            li = local.setdefault(r, len(local))
            b_, m = divmod(s, P)
            seg[m, b_ * P + li] = 1.0
            segT[li, b_ * P + m] = 1.0
        in_maps.append({"qp": qp, "msk": msk, "seg": seg, "segT": segT})
    return in_maps, cols_of, (slots, Wg, NB)


def _build_nc(layout, reps=1):
    from concourse import bacc, bass, tile

    mybir = bass.mybir
    dt = mybir.dt.float32
    dtb = mybir.dt.bfloat16
    Alu = mybir.AluOpType
    ActF = mybir.ActivationFunctionType
    slots, Wg, NB = layout
    ncol = len(slots)
    COLS = NB * P
    SUMW = sum(Wg)
    off_g = [0]
    for w in Wg:
        off_g.append(off_g[-1] + w)

    nc = bacc.Bacc("TRN2", target_bir_lowering=False, debug=False,
                   num_devices=NCORES)
    qp = nc.declare_dram_parameter("qp", [P, SUMW], dtb, isOutput=False)
    msk = nc.declare_dram_parameter("msk", [P, COLS], dt, isOutput=False)
    seg = nc.declare_dram_parameter("seg", [P, COLS], dt, isOutput=False)
    segT = nc.declare_dram_parameter("segT", [P, COLS], dt, isOutput=False)
    iden = nc.declare_dram_parameter("iden", [P, P], dt, isOutput=False)
    wm = nc.declare_dram_parameter("wm", [E, E], dt, isOutput=False)
    wv = nc.declare_dram_parameter("wv", [4, P], dt, isOutput=False)
    # shape varies with reps so the jax persistent compile cache cannot
    # alias NEFFs of different-reps builds (the BIR is not in the HLO key)
    nc.declare_dram_parameter("stamp", [1, reps], dt, isOutput=False)
    probs = nc.declare_dram_parameter("probs", [COLS, P], dt, isOutput=True)

    with tile.TileContext(nc) as tc:
        with (
            tc.tile_pool(name="const", bufs=1) as const,
            tc.tile_pool(name="qpool", bufs=6) as qpool,
            tc.tile_pool(name="spool", bufs=NB + 2) as spool,
            tc.tile_pool(name="scratch", bufs=4) as scratch,
            tc.tile_pool(name="rpool", bufs=2) as rpool,
            tc.tile_pool(name="ppool", bufs=2) as ppool,
            tc.tile_pool(name="psum", bufs=1, space=bass.MemorySpace.PSUM) as psp,
            tc.tile_pool(name="psum2", bufs=2, space=bass.MemorySpace.PSUM) as psp2,
            tc.tile_pool(name="psc", bufs=2, space=bass.MemorySpace.PSUM) as psc,
        ):
            iden_sb = const.tile([P, P], dt, tag="iden")
            nc.sync.dma_start(iden_sb[:], iden[:])
            msk_sb = const.tile([P, COLS], dt, tag="msk")
            nc.sync.dma_start(msk_sb[:], msk[:])
            seg_sb = const.tile([P, COLS], dt, tag="seg")
            nc.sync.dma_start(seg_sb[:], seg[:])
            segT_sb = const.tile([P, COLS], dt, tag="segT")
            nc.sync.dma_start(segT_sb[:], segT[:])
            w_sb = const.tile([P, 4 * E], dt, tag="wmat")
            for j in range(4):
                nc.sync.dma_start(w_sb[:, j * E:(j + 1) * E],
                                  wm[j * P:(j + 1) * P, :])
            wv4 = const.tile([4, P], dt, tag="wv4")
            nc.sync.dma_start(wv4[:], wv[:])

            # wvT[f_in_chunk, fchunk] = wv[f] on 128 partitions
            wvT_ps = psp.tile([P, 4], dt, tag="setup")
            nc.tensor.transpose(wvT_ps[:], wv4[:], iden_sb[0:4, 0:4])
            wvT_sb = const.tile([P, 4], dt, tag="wvT")
            nc.scalar.copy(wvT_sb[:], wvT_ps[:])
            # vT[e_in_chunk, echunk] = (W.T @ wv)[e], via 4x4 accumulating
            # matvecs: out[m, c] = sum_f W[f, c*128+m] * wv[f]
            vT_ps = psp.tile([P, CH], dt, tag="setup")
            with tc.tile_critical():
                for cch in range(CH):
                    for j in range(4):
                        nc.tensor.matmul(
                            vT_ps[:, cch:cch + 1],
                            w_sb[:, j * E + cch * P:j * E + (cch + 1) * P],
                            wvT_sb[:, j:j + 1],
                            start=(j == 0), stop=(j == 3))
            vT_bf = const.tile([P, CH], dtb, tag="vTbf")
            nc.vector.tensor_copy(vT_bf[:], vT_ps[:])

            # The tail of pass r (transpose -> exp -> segment row-sums ->
            # normalize -> store) is a serial cross-engine latency chain;
            # its stages are emitted interleaved into pass r+1's matmul
            # stream so every semaphore wait hides under DMA/PE streaming.
            def tail_block(sc2_b, b):
                # (walrus: transpose matmul output must start at PSUM
                # partition 0, so each 64-col chunk gets its own tile)
                pr_b = ppool.tile([P, P], dt, tag=f"pr{b}")
                se_b = ppool.tile([P, 1], dt, tag=f"se{b}")
                for i in range(2):
                    pt_ps = psp2.tile([64, P], dt, tag="tps")
                    nc.tensor.transpose(pt_ps[:],
                                        sc2_b[:, 64 * i:64 * (i + 1)],
                                        iden_sb[:])
                    nc.scalar.activation(
                        pr_b[64 * i:64 * (i + 1), :],
                        pt_ps[:],
                        ActF.Exp,
                        accum_out=se_b[64 * i:64 * (i + 1), 0:1])
                return pr_b, se_b

            def tail_sums(parts):
                rowsum_parts, blocks = parts
                for b, (pr_b, se_b) in enumerate(blocks):
                    rs_ps = psp2.tile([P, 1], dt, tag="v1")
                    nc.tensor.matmul(rs_ps[:], seg_sb[:, b * P:(b + 1) * P],
                                     se_b[:], start=True, stop=True)
                    nc.scalar.copy(rowsum_parts[:, b:b + 1], rs_ps[:])

            def tail_norm(parts):
                rowsum_parts, blocks = parts
                rowsum = rpool.tile([P, 1], dt, tag="rowsum")
                nc.vector.reduce_sum(rowsum[:], rowsum_parts[:],
                                     axis=mybir.AxisListType.X)
                rowsum_eps = rpool.tile([P, 1], dt, tag="rowsum_eps")
                # keep unused-row reciprocals finite so 0-weight matmul
                # terms stay 0 instead of 0*inf
                nc.vector.tensor_scalar_add(rowsum_eps[:], rowsum[:], 1e-30)
                recip_sb = rpool.tile([P, 1], dt, tag="recip")
                nc.vector.reciprocal(recip_sb[:], rowsum_eps[:])
                for b, (pr_b, se_b) in enumerate(blocks):
                    ex_ps = psp2.tile([P, 1], dt, tag="v1")
                    nc.tensor.matmul(ex_ps[:], segT_sb[:, b * P:(b + 1) * P],
                                     recip_sb[:], start=True, stop=True)
                    ex_sb = scratch.tile([P, 1], dt, tag="exs")
                    nc.scalar.copy(ex_sb[:], ex_ps[:])
                    fin = scratch.tile([P, P], dt, tag="fin")
                    nc.vector.tensor_scalar_mul(fin[:], pr_b[:], ex_sb[:])
                    nc.sync.dma_start(probs[b * P:(b + 1) * P, :], fin[:])

            def mm_phase(deferred):
                """Emit this pass's matmuls; interleave the previous
                pass's tail stages between blocks. Returns this pass's
                sc2 tiles for its own deferred tail."""
                live = {}
                sc2s = []
                for b in range(NB):
                    sc_ps = psc.tile([P, P], dt, tag="scps")
                    # dummy cols / tokens past a col's width keep NEG; a
                    # col's matmul group overwrites only rows [0, w)
                    nc.vector.memset(sc_ps[:], NEG)
                    for s in range(b * P, min((b + 1) * P, ncol)):
                        g, off, w = slots[s]
                        cl = s - b * P
                        if g not in live:
                            qt = qpool.tile([P, WMAX], dtb, tag="q")
                            nc.sync.dma_start(qt[:, :Wg[g]],
                                              qp[:, off_g[g]:off_g[g + 1]])
                            live[g] = qt
                        qt = live[g]
                        for j in range(CH):
                            nc.tensor.matmul(
                                sc_ps[0:w, cl:cl + 1],
                                qt[:, off + j * w:off + (j + 1) * w],
                                vT_bf[:, j:j + 1],
                                start=(j == 0), stop=(j == CH - 1))
                    # apply length/padding mask
                    sc2_b = spool.tile([P, P], dt, tag="scores2")
                    nc.vector.tensor_tensor(
                        out=sc2_b[:], in0=sc_ps[:],
                        in1=msk_sb[:, b * P:(b + 1) * P], op=Alu.add)
                    sc2s.append(sc2_b)
                    if deferred is not None:
                        deferred[b]()
                if deferred is not None:
                    deferred[NB]()
                return sc2s

            def make_deferred(sc2s):
                rowsum_parts = rpool.tile([P, NB], dt, tag="rsparts")
                parts = (rowsum_parts, [])

                def block_stage(b):
                    return lambda: parts[1].append(tail_block(sc2s[b], b))

                def sums_stage():
                    tail_sums(parts)
                    tail_norm(parts)
                return [block_stage(b) for b in range(NB)] + [sums_stage]

            deferred = None
            for _rep in range(reps):
                sc2s = mm_phase(deferred)
                deferred = make_deferred(sc2s)
            for stage in deferred:
                stage()

    nc.compile()
    return nc


def kernel(**inputs):
    global LAST_RESULT
    from concourse.bass_utils import run_bass_kernel_spmd

    questions = np.ascontiguousarray(np.asarray(inputs["questions"], np.float32))
    lens = np.asarray(inputs["questions_lens"], np.int32)
    W = np.ascontiguousarray(np.asarray(inputs["W"], np.float32))
    wv = np.ascontiguousarray(np.asarray(inputs["weight_vec"], np.float32))
    B2, L, E_ = questions.shape

    in_maps, cols_of, layout = _pack(questions, lens, NCORES)
    iden = np.eye(P, dtype=np.float32)
    wvr = np.ascontiguousarray(wv.reshape(4, P))
    for m in in_maps:
        m["iden"] = iden
        m["wm"] = W
        m["wv"] = wvr
        m["stamp"] = np.zeros((1, 1), np.float32)

    if layout not in _NC_CACHE:
        _NC_CACHE[layout] = _build_nc(layout)
    nc = _NC_CACHE[layout]

    res = run_bass_kernel_spmd(nc, in_maps, list(range(NCORES)))
    LAST_RESULT = res

    out = np.zeros((B2, L), np.float32)
    for c in range(NCORES):
        pr = res.results[c]["probs"]
        for s, col in enumerate(cols_of[c]):
            if col is None:
                continue
            r, t, ntok = col
            out[r, t * P:t * P + ntok] = pr[s, :ntok]
    return out


# revision 12
# speedup vs baseline: 1.9358x; 1.4893x over previous
"""Ragged masked-softmax attention-energy kernel for 8 Trainium2 NeuronCores.

Reference computation (B2=512, L=1024, E=512):
    energy = questions @ W.T + b              [B2, L, E]
    scores = energy @ weight_vec              [B2, L]
    scores[l >= len] = -inf
    out = softmax(scores, axis=1)

Algebraic facts that make this a pure HBM-streaming problem:
  * (q @ W.T + b) @ wv == q @ (W.T @ wv) + (b . wv); softmax is shift
    invariant so the (b . wv) scalar cancels. Only v = W.T @ wv (a [E]
    vector, computed on device) ever multiplies the big tensor.
  * tokens at positions >= len contribute exactly 0 to the output, so
    only the first `len` tokens of each row need to be loaded at all.

The big tensor is packed in bf16 (tolerance is 2e-2; bf16 costs ~0.1%
of a prob) and TRANSPOSED on the host into [e, tok] column tiles of
EXACT width (no padding to 128): each column holds w <= 128 tokens of
one row, stored as 4 e-chunks of [128, w]. The per-token dot products
run on the TensorEngine: per column, four accumulating matmuls with the
q e-chunk as the (FWL-eligible bf16) stationary operand and the
matching vT chunk as a 1-column moving operand. Scores land directly as
[tok, col] fp32 PSUM blocks; the tail per 128-column block is mask add,
PE transpose, ACT exp with free-axis accumulate (per-col sums), and 0/1
segment matmuls that map columns to rows entirely as data, keeping the
SPMD program identical on all 8 cores. Column slot widths are the
elementwise max over cores of each core's width-sorted column list, so
the program is uniform while wasting <1% over the exact token count.

Host side is pure data layout (transpose/cast/scatter, no math): rows
are LPT-packed across cores by exact token count; host scatters the
packed [col, 128] probabilities back into the [B2, L] output.
"""

import os
import sys

import numpy as np
import ml_dtypes

if "/opt/trn_rl_repo" not in sys.path:
    sys.path.insert(0, "/opt/trn_rl_repo")

E = 512
P = 128
CH = E // P  # e-chunks per tile
WMAX = 8192  # free-dim elements per DMA group (= 2 MiB bf16 per group)
NCORES = 8
NEG = -1.0e30
BF16 = ml_dtypes.bfloat16

_NC_CACHE = {}
LAST_RESULT = None


def _schedule(lens, n_cores):
    """Assign rows to cores (LPT by exact token count, <=128 rows/core)."""
    order = sorted(range(len(lens)), key=lambda r: -int(lens[r]))
    loads = [0] * n_cores
    rows_of = [[] for _ in range(n_cores)]
    for r in order:
        cands = [c for c in range(n_cores) if len(rows_of[c]) < P]
        c = min(cands, key=lambda i: (loads[i], len(rows_of[i])))
        rows_of[c].append(r)
        loads[c] += int(lens[r])
    return rows_of


def _layout(lens, n_cores):
    """Uniform SPMD layout: per-core width-sorted columns, slot widths =
    elementwise max over cores, greedy grouping into <=WMAX DMA groups.

    Returns (slots, Wg, NB, cols_of): slots[i] = (g, off, w); Wg[g] =
    used free width of group g; cols_of[c][i] = (row, tile, ntok) or
    None for this core's dummy slots.
    """
    rows_of = _schedule(lens, n_cores)
    percore = []
    for c in range(n_cores):
        cols = []
        for r in rows_of[c]:
            L = int(lens[r])
            for t in range((L + P - 1) // P):
                cols.append((min(P, L - t * P), r, t))
        cols.sort(key=lambda x: (-x[0], x[1], x[2]))
        percore.append(cols)
    ncol = max(len(c) for c in percore)
    wslot = []
    for i in range(ncol):
        w = max((c[i][0] if i < len(c) else 0) for c in percore)
        wslot.append(min(P, ((w + 1) // 2) * 2))
    slots = []
    Wg = []
    cur = 0
    for i, w in enumerate(wslot):
        need = CH * w
        if cur + need > WMAX:
            Wg.append(cur)
            cur = 0
        slots.append((len(Wg), cur, w))
        cur += need
    Wg.append(cur)
    NB = -(-ncol // P)
    cols_of = [[(c[i][1], c[i][2], c[i][0]) if i < len(c) else None
                for i in range(ncol)] for c in percore]
    return tuple(slots), tuple(Wg), NB, cols_of


def _pack(questions, lens, n_cores):
    B2, L, E_ = questions.shape
    assert E_ == E
    slots, Wg, NB, cols_of = _layout(lens, n_cores)
    ncol = len(slots)
    COLS = NB * P
    SUMW = sum(Wg)
    off_g = np.concatenate([[0], np.cumsum(Wg)]).astype(np.int64)
    in_maps = []
    for c in range(n_cores):
        qp = np.zeros((P, SUMW), BF16)
        msk = np.full((P, COLS), NEG, np.float32)
        seg = np.zeros((P, COLS), np.float32)
        segT = np.zeros((P, COLS), np.float32)
        local = {}
        for s in range(ncol):
            if cols_of[c][s] is None:
                continue
            r, t, ntok = cols_of[c][s]
            g, off, w = slots[s]
            base = off_g[g] + off
            qt = questions[r, t * P:t * P + ntok, :]  # [ntok, E] fp32
            for j in range(CH):  # [e_in_chunk, tok] per chunk at stride w
                qp[:, base + j * w:base + j * w + ntok] = qt[:, j * P:(j + 1) * P].T
            msk[:ntok, s] = 0.0
            li = local.setdefault(r, len(local))
            b_, m = divmod(s, P)
            seg[m, b_ * P + li] = 1.0
            segT[li, b_ * P + m] = 1.0
        in_maps.append({"qp": qp, "msk": msk, "seg": seg, "segT": segT})
    return in_maps, cols_of, (slots, Wg, NB)


def _build_nc(layout, reps=1):
    from concourse import bacc, bass, tile

    mybir = bass.mybir
    dt = mybir.dt.float32
    dtb = mybir.dt.bfloat16
    Alu = mybir.AluOpType
    ActF = mybir.ActivationFunctionType
    slots, Wg, NB = layout
    ncol = len(slots)
    COLS = NB * P
    SUMW = sum(Wg)
    off_g = [0]
    for w in Wg:
        off_g.append(off_g[-1] + w)

    nc = bacc.Bacc("TRN2", target_bir_lowering=False, debug=False,
                   num_devices=NCORES)
    qp = nc.declare_dram_parameter("qp", [P, SUMW], dtb, isOutput=False)
    msk = nc.declare_dram_parameter("msk", [P, COLS], dt, isOutput=False)
    seg = nc.declare_dram_parameter("seg", [P, COLS], dt, isOutput=False)
    segT = nc.declare_dram_parameter("segT", [P, COLS], dt, isOutput=False)
    iden = nc.declare_dram_parameter("iden", [P, P], dt, isOutput=False)
    wm = nc.declare_dram_parameter("wm", [E, E], dt, isOutput=False)
    wv = nc.declare_dram_parameter("wv", [4, P], dt, isOutput=False)
    # shape varies with reps so the jax persistent compile cache cannot
    # alias NEFFs of different-reps builds (the BIR is not in the HLO key)
    nc.declare_dram_parameter("stamp", [1, reps], dt, isOutput=False)
    probs = nc.declare_dram_parameter("probs", [COLS, P], dt, isOutput=True)

    with tile.TileContext(nc) as tc:
        with (
            tc.tile_pool(name="const", bufs=1) as const,
            tc.tile_pool(name="qpool", bufs=8) as qpool,
            tc.tile_pool(name="spool", bufs=NB + 2) as spool,
            tc.tile_pool(name="scratch", bufs=4) as scratch,
            tc.tile_pool(name="rpool", bufs=2) as rpool,
            tc.tile_pool(name="ppool", bufs=2) as ppool,
            tc.tile_pool(name="psum", bufs=1, space=bass.MemorySpace.PSUM) as psp,
            tc.tile_pool(name="psum2", bufs=2, space=bass.MemorySpace.PSUM) as psp2,
            tc.tile_pool(name="psc", bufs=2, space=bass.MemorySpace.PSUM) as psc,
        ):
            iden_sb = const.tile([P, P], dt, tag="iden")
            nc.sync.dma_start(iden_sb[:], iden[:])
            msk_sb = const.tile([P, COLS], dt, tag="msk")
            nc.sync.dma_start(msk_sb[:], msk[:])
            seg_sb = const.tile([P, COLS], dt, tag="seg")
            nc.sync.dma_start(seg_sb[:], seg[:])
            segT_sb = const.tile([P, COLS], dt, tag="segT")
            nc.sync.dma_start(segT_sb[:], segT[:])
            w_sb = const.tile([P, 4 * E], dt, tag="wmat")
            for j in range(4):
                nc.sync.dma_start(w_sb[:, j * E:(j + 1) * E],
                                  wm[j * P:(j + 1) * P, :])
            wv4 = const.tile([4, P], dt, tag="wv4")
            nc.sync.dma_start(wv4[:], wv[:])

            # wvT[f_in_chunk, fchunk] = wv[f] on 128 partitions
            wvT_ps = psp.tile([P, 4], dt, tag="setup")
            nc.tensor.transpose(wvT_ps[:], wv4[:], iden_sb[0:4, 0:4])
            wvT_sb = const.tile([P, 4], dt, tag="wvT")
            nc.scalar.copy(wvT_sb[:], wvT_ps[:])
            # vT[e_in_chunk, echunk] = (W.T @ wv)[e], via 4x4 accumulating
            # matvecs: out[m, c] = sum_f W[f, c*128+m] * wv[f]
            vT_ps = psp.tile([P, CH], dt, tag="setup")
            with tc.tile_critical():
                for cch in range(CH):
                    for j in range(4):
                        nc.tensor.matmul(
                            vT_ps[:, cch:cch + 1],
                            w_sb[:, j * E + cch * P:j * E + (cch + 1) * P],
                            wvT_sb[:, j:j + 1],
                            start=(j == 0), stop=(j == 3))
            vT_bf = const.tile([P, CH], dtb, tag="vTbf")
            nc.vector.tensor_copy(vT_bf[:], vT_ps[:])

            # The tail of pass r (transpose -> exp -> segment row-sums ->
            # normalize -> store) is a serial cross-engine latency chain;
            # its stages are emitted interleaved into pass r+1's matmul
            # stream so every semaphore wait hides under DMA/PE streaming.
            def tail_block(sc2_b, b):
                # (walrus: transpose matmul output must start at PSUM
                # partition 0, so each 64-col chunk gets its own tile)
                pr_b = ppool.tile([P, P], dt, tag=f"pr{b}")
                se_b = ppool.tile([P, 1], dt, tag=f"se{b}")
                for i in range(2):
                    pt_ps = psp2.tile([64, P], dt, tag="tps")
                    nc.tensor.transpose(pt_ps[:],
                                        sc2_b[:, 64 * i:64 * (i + 1)],
                                        iden_sb[:])
                    nc.scalar.activation(
                        pr_b[64 * i:64 * (i + 1), :],
                        pt_ps[:],
                        ActF.Exp,
                        accum_out=se_b[64 * i:64 * (i + 1), 0:1])
                return pr_b, se_b

            def tail_sum(parts, b):
                rowsum_parts, blocks = parts
                se_b = blocks[b][1]
                rs_ps = psp2.tile([P, 1], dt, tag="v1")
                nc.tensor.matmul(rs_ps[:], seg_sb[:, b * P:(b + 1) * P],
                                 se_b[:], start=True, stop=True)
                nc.scalar.copy(rowsum_parts[:, b:b + 1], rs_ps[:])

            def tail_norm(parts):
                rowsum_parts, blocks = parts
                rowsum = rpool.tile([P, 1], dt, tag="rowsum")
                nc.vector.reduce_sum(rowsum[:], rowsum_parts[:],
                                     axis=mybir.AxisListType.X)
                rowsum_eps = rpool.tile([P, 1], dt, tag="rowsum_eps")
                # keep unused-row reciprocals finite so 0-weight matmul
                # terms stay 0 instead of 0*inf
                nc.vector.tensor_scalar_add(rowsum_eps[:], rowsum[:], 1e-30)
                recip_sb = rpool.tile([P, 1], dt, tag="recip")
                nc.vector.reciprocal(recip_sb[:], rowsum_eps[:])
                for b, (pr_b, se_b) in enumerate(blocks):
                    ex_ps = psp2.tile([P, 1], dt, tag="v1")
                    nc.tensor.matmul(ex_ps[:], segT_sb[:, b * P:(b + 1) * P],
                                     recip_sb[:], start=True, stop=True)
                    ex_sb = scratch.tile([P, 1], dt, tag="exs")
                    nc.scalar.copy(ex_sb[:], ex_ps[:])
                    fin = scratch.tile([P, P], dt, tag="fin")
                    nc.vector.tensor_scalar_mul(fin[:], pr_b[:], ex_sb[:])
                    nc.sync.dma_start(probs[b * P:(b + 1) * P, :], fin[:])

            def mm_phase(deferred):
                """Emit this pass's matmuls; interleave the previous
                pass's tail stages between blocks. Returns this pass's
                sc2 tiles for its own deferred tail."""
                live = {}
                sc2s = []
                for b in range(NB):
                    sc_ps = psc.tile([P, P], dt, tag="scps")
                    # dummy cols / tokens past a col's width keep NEG; a
                    # col's matmul group overwrites only rows [0, w)
                    nc.vector.memset(sc_ps[:], NEG)
                    for s in range(b * P, min((b + 1) * P, ncol)):
                        g, off, w = slots[s]
                        cl = s - b * P
                        if g not in live:
                            qt = qpool.tile([P, WMAX], dtb, tag="q")
                            nc.sync.dma_start(qt[:, :Wg[g]],
                                              qp[:, off_g[g]:off_g[g + 1]])
                            live[g] = qt
                        qt = live[g]
                        for j in range(CH):
                            nc.tensor.matmul(
                                sc_ps[0:w, cl:cl + 1],
                                qt[:, off + j * w:off + (j + 1) * w],
                                vT_bf[:, j:j + 1],
                                start=(j == 0), stop=(j == CH - 1))
                    # apply length/padding mask
                    sc2_b = spool.tile([P, P], dt, tag="scores2")
                    nc.vector.tensor_tensor(
                        out=sc2_b[:], in0=sc_ps[:],
                        in1=msk_sb[:, b * P:(b + 1) * P], op=Alu.add)
                    sc2s.append(sc2_b)
                    if deferred is not None:
                        deferred[b]()
                if deferred is not None:
                    deferred[NB]()
                return sc2s

            def make_deferred(sc2s):
                rowsum_parts = rpool.tile([P, NB], dt, tag="rsparts")
                parts = (rowsum_parts, [])

                def block_stage(b):
                    def run():
                        parts[1].append(tail_block(sc2s[b], b))
                        if b > 0:
                            tail_sum(parts, b - 1)
                    return run

                def norm_stage():
                    tail_sum(parts, NB - 1)
                    tail_norm(parts)
                return [block_stage(b) for b in range(NB)] + [norm_stage]

            deferred = None
            for _rep in range(reps):
                sc2s = mm_phase(deferred)
                deferred = make_deferred(sc2s)
            for stage in deferred:
                stage()

    nc.compile()
    return nc


def kernel(**inputs):
    global LAST_RESULT
    from concourse.bass_utils import run_bass_kernel_spmd

    questions = np.ascontiguousarray(np.asarray(inputs["questions"], np.float32))
    lens = np.asarray(inputs["questions_lens"], np.int32)
    W = np.ascontiguousarray(np.asarray(inputs["W"], np.float32))
    wv = np.ascontiguousarray(np.asarray(inputs["weight_vec"], np.float32))
    B2, L, E_ = questions.shape

    in_maps, cols_of, layout = _pack(questions, lens, NCORES)
    iden = np.eye(P, dtype=np.float32)
    wvr = np.ascontiguousarray(wv.reshape(4, P))
    for m in in_maps:
        m["iden"] = iden
        m["wm"] = W
        m["wv"] = wvr
        m["stamp"] = np.zeros((1, 1), np.float32)

    if layout not in _NC_CACHE:
        _NC_CACHE[layout] = _build_nc(layout)
    nc = _NC_CACHE[layout]

    res = run_bass_kernel_spmd(nc, in_maps, list(range(NCORES)))
    LAST_RESULT = res

    out = np.zeros((B2, L), np.float32)
    for c in range(NCORES):
        pr = res.results[c]["probs"]
        for s, col in enumerate(cols_of[c]):
            if col is None:
                continue
            r, t, ntok = col
            out[r, t * P:t * P + ntok] = pr[s, :ntok]
    return out
